# revision 2
# baseline (speedup 1.0000x reference)
"""MiniBatchDiscrimination Trainium2 kernel.

reference:
    M = einsum('nhwf,fbc->nhwbc', x, T)          # [N,H,W,B,C]
    norm = sum_c |M[i] - M[j]|                   # [N,N,H,W,B]
    o_b  = sum_j exp(-norm)                      # [N,H,W,B]
    out  = concat([x, o_b], axis=3)              # [N,H,W,F+B]

Sharding: data-parallel over the outer batch axis N (4 rows per core, 8
cores); every core receives the full x (as a pre-transposed fp16 copy) and
computes the whole M on-chip, then only its 4 rows of the pairwise block.

Per-core device layout ("L2"): M2_q [(b16,c8) partitions, (n32,hw256) free]
for each b-quarter q, so that
  - M-compute is a plain matmul (lhsT = T-tile [f,(b,c)], rhs = xT [f,(n,hw)])
  - the pairwise |M_j - M_i| is a DVE tensor op between free-dim slices
    (j-block read vs broadcast i-slice)
  - the c-reduction contracts the partition axis on the TensorEngine with
    stripe-ones matrices, accumulating 8 (q',i) stripes into one PSUM tile
    [(q',i,b) partitions, (j,hw) free]
  - exp(-norm) is one ACT pass, and the j-sum is a windowed DVE reduce
    over the strided j axis.
"""

import os
import sys

for _p in ("/opt/trn_rl_repo", "/opt/pypackages"):
    if _p not in sys.path and os.path.isdir(_p):
        sys.path.append(_p)

import numpy as np

N, HW, F, B, C = 32, 256, 256, 64, 8
NL = 4          # local rows per core
CORES = 8
FH = 2          # f in two partition halves of 128
Q = 4           # b-quarters of 16
HWC = 4         # hw chunks of 64
HW_CH = HW // HWC

F16 = "float16"


# --------------------------------------------------------------------------
# device program
# --------------------------------------------------------------------------

def build_body(tc, outs, ins):
    """Trace the per-core Tile program.

    ins:  xT   [2,128,8192] f16   xT[fh,f,n*256+hw] = x[n,hw,fh*128+f]
          xiT  [2,128,1024] f16   same, restricted to this core's 4 rows
          tw   [2,4,128,128] f16  tw[fh,q,f,b*8+c] = T[fh*128+f,16q+b,c]
          ones [8,128,128]  f16   ones[s,b*8+c,col] = (col == 16s+b)
    outs: o    [2,128,256]  f32   o[t, 64q'+16i+b, hw] = o_b[ib+i, hw, 16(2t+q')+b]
    """
    from contextlib import ExitStack

    import concourse.bass as bass
    import concourse.mybir as mybir

    nc = tc.nc
    f16 = mybir.dt.float16
    f32 = mybir.dt.float32

    xT_d, xiT_d, tw_d, ones_d = ins["xT"], ins["xiT"], ins["tw"], ins["ones"]
    o_d = outs["o"]

    with ExitStack() as ctx:
        singles = ctx.enter_context(tc.tile_pool(name="singles", bufs=1))
        psA = ctx.enter_context(tc.tile_pool(name="psA", bufs=2, space="PSUM"))
        psN = ctx.enter_context(tc.tile_pool(name="psN", bufs=2, space="PSUM"))
        adp = ctx.enter_context(tc.tile_pool(name="adp", bufs=10))
        Ep = ctx.enter_context(tc.tile_pool(name="Ep", bufs=2))

        # ---- loads -------------------------------------------------------
        xT_s, xiT_s, tw_s = [], [], []
        for fh in range(FH):
            t = singles.tile([128, N * HW], f16, tag=f"xT{fh}")
            nc.sync.dma_start(out=t, in_=xT_d[fh])
            xT_s.append(t)
            t = singles.tile([128, NL * HW], f16, tag=f"xiT{fh}")
            nc.sync.dma_start(out=t, in_=xiT_d[fh])
            xiT_s.append(t)
            row = []
            for q in range(Q):
                t = singles.tile([128, 128], f16, tag=f"tw{fh}{q}")
                nc.sync.dma_start(out=t, in_=tw_d[fh, q])
                row.append(t)
            tw_s.append(row)
        ones_s = []
        for s in range(8):
            t = singles.tile([128, 128], f16, tag=f"ones{s}")
            nc.sync.dma_start(out=t, in_=ones_d[s])
            ones_s.append(t)

        # ---- stage B: M2 = T' @ xT  (and M2i from the local rows) --------
        M2, M2i = [], []
        for q in range(Q):
            m2 = singles.tile([128, N * HW], f16, tag=f"m2{q}")
            for piece in range(8):          # 1024-col pieces
                ps = psA.tile([128, 1024], f32, tag="psA")
                for sub in range(2):        # 512-col matmuls
                    sl = slice(sub * 512, (sub + 1) * 512)
                    src = slice(piece * 1024 + sub * 512, piece * 1024 + (sub + 1) * 512)
                    for fh in range(FH):
                        nc.tensor.matmul(
                            ps[:, sl], lhsT=tw_s[fh][q], rhs=xT_s[fh][:, src],
                            start=(fh == 0), stop=(fh == 1),
                        )
                nc.scalar.copy(out=m2[:, piece * 1024:(piece + 1) * 1024], in_=ps[:])
            M2.append(m2)
        for q in range(Q):
            m2i = singles.tile([128, NL * HW], f16, tag=f"m2i{q}")
            ps = psA.tile([128, 1024], f32, tag="psA")
            for sub in range(2):
                sl = slice(sub * 512, (sub + 1) * 512)
                for fh in range(FH):
                    nc.tensor.matmul(
                        ps[:, sl], lhsT=tw_s[fh][q], rhs=xiT_s[fh][:, sl],
                        start=(fh == 0), stop=(fh == 1),
                    )
            nc.scalar.copy(out=m2i, in_=ps[:])
            M2i.append(m2i)

        # ---- stage C: pairwise |diff|, c-reduce, exp, j-sum --------------
        for t in range(2):                  # b-quarter pairs (q = 2t+q')
            o_sb = singles.tile([128, HW], f32, tag=f"osb{t}")
            for hwc in range(HWC):
                hsl = slice(hwc * HW_CH, (hwc + 1) * HW_CH)
                ads = []
                for qp in range(2):
                    q = 2 * t + qp
                    m2v = M2[q].rearrange("p (n hw) -> p n hw", n=N)
                    m2iv = M2i[q].rearrange("p (i hw) -> p i hw", i=NL)
                    for i in range(NL):
                        ad = adp.tile([128, N * HW_CH], f16, tag="ad")
                        adv = ad.rearrange("p (n hw) -> p n hw", n=N)
                        src0 = m2v[:, :, hsl]
                        s1 = m2iv[:, i, hsl]           # [128, 64]
                        src1 = bass.AP(
                            tensor=s1.tensor, offset=s1.offset,
                            ap=[list(s1.ap[0]), [0, N], list(s1.ap[1])],
                        )
                        nc.vector.tensor_sub(adv, src0, src1)
                        nc.vector.scalar_tensor_tensor(
                            out=ad, in0=ad, scalar=-1.0, in1=ad,
                            op0=mybir.AluOpType.mult, op1=mybir.AluOpType.max,
                        )
                        ads.append(ad)
                E = Ep.tile([128, N * HW_CH], f32, tag="E")
                for h in range(2):          # two 1024-col norm tiles
                    nrm = psN.tile([128, 1024], f32, tag="nrm")
                    for sub in range(2):
                        sl = slice(sub * 512, (sub + 1) * 512)
                        cc = slice(h * 1024 + sub * 512, h * 1024 + (sub + 1) * 512)
                        for s in range(8):
                            nc.tensor.matmul(
                                nrm[:, sl], lhsT=ones_s[s], rhs=ads[s][:, cc],
                                start=(s == 0), stop=(s == 7),
                            )
                    nc.scalar.activation(
                        out=E[:, h * 1024:(h + 1) * 1024], in_=nrm[:],
                        func=mybir.ActivationFunctionType.Exp, scale=-1.0,
                    )
                Ev = E.rearrange("p (j hw) -> p hw j", j=N)
                nc.vector.tensor_reduce(
                    out=o_sb[:, hsl], in_=Ev,
                    axis=mybir.AxisListType.X, op=mybir.AluOpType.add,
                )
            nc.sync.dma_start(out=o_d[t], in_=o_sb)


# --------------------------------------------------------------------------
# host side
# --------------------------------------------------------------------------

def prep_inputs(x, T):
    """Shared (core-independent) device inputs."""
    xf = np.ascontiguousarray(x.reshape(N, HW, F))
    xT_np = np.ascontiguousarray(xf.transpose(2, 0, 1).reshape(F, N * HW))
    xT_in = xT_np.reshape(FH, 128, N * HW).astype(np.float16)
    tw = T.reshape(FH, 128, Q, 16, C).transpose(0, 2, 1, 3, 4)
    tw_in = np.ascontiguousarray(tw.reshape(FH, Q, 128, 128)).astype(np.float16)
    ones_in = np.zeros((8, 128, 128), np.float16)
    for s in range(8):
        for b in range(16):
            ones_in[s, b * 8:(b + 1) * 8, 16 * s + b] = 1.0
    return xT_np, xT_in, tw_in, ones_in


def core_in_map(xT_np, xT_in, tw_in, ones_in, k):
    xiT = np.ascontiguousarray(
        xT_np[:, k * NL * HW:(k + 1) * NL * HW]
    ).reshape(FH, 128, NL * HW).astype(np.float16)
    return {"xT": xT_in, "xiT": xiT, "tw": tw_in, "ones": ones_in}


def gather_ob(core_outs):
    """core_outs: list of 8 arrays [2,128,256] f32 -> o_b [N,16,16,B]."""
    obs = []
    for res in core_outs:
        v = res.reshape(2, 2, NL, 16, HW)          # t, q', i, b, hw
        obs.append(v.transpose(2, 4, 0, 1, 3).reshape(NL, HW, B))
    return np.concatenate(obs, axis=0).reshape(N, 16, 16, B)


_CACHED = {}


def _get_program():
    if "nc" in _CACHED:
        return _CACHED["nc"]
    from contextlib import ExitStack

    import concourse.bacc as bacc
    import concourse.mybir as mybir
    import concourse.tile as tile

    nc = bacc.Bacc("TRN2", target_bir_lowering=False, debug=False,
                   num_devices=CORES)
    f16, f32 = mybir.dt.float16, mybir.dt.float32
    ins = {
        "xT": nc.dram_tensor("xT", [FH, 128, N * HW], f16, kind="ExternalInput").ap(),
        "xiT": nc.dram_tensor("xiT", [FH, 128, NL * HW], f16, kind="ExternalInput").ap(),
        "tw": nc.dram_tensor("tw", [FH, Q, 128, 128], f16, kind="ExternalInput").ap(),
        "ones": nc.dram_tensor("ones", [8, 128, 128], f16, kind="ExternalInput").ap(),
    }
    outs = {
        "o": nc.dram_tensor("o", [2, 128, HW], f32, kind="ExternalOutput").ap(),
    }
    with tile.TileContext(nc) as tc:
        build_body(tc, outs, ins)
    nc.compile()
    _CACHED["nc"] = nc
    return nc


def kernel(x, T):
    x = np.asarray(x, dtype=np.float32)
    T = np.asarray(T, dtype=np.float32)
    from concourse.bass_utils import run_bass_kernel_spmd

    nc = _get_program()
    xT_np, xT_in, tw_in, ones_in = prep_inputs(x, T)
    in_maps = [core_in_map(xT_np, xT_in, tw_in, ones_in, k) for k in range(CORES)]
    res = run_bass_kernel_spmd(nc, in_maps, core_ids=list(range(CORES)))
    ob = gather_ob([r["o"] for r in res.results])
    return np.concatenate([x, ob], axis=3)


# revision 4
# speedup vs baseline: 1.0186x; 1.0186x over previous
"""MiniBatchDiscrimination Trainium2 kernel.

reference:
    M = einsum('nhwf,fbc->nhwbc', x, T)          # [N,H,W,B,C]
    norm = sum_c |M[i] - M[j]|                   # [N,N,H,W,B]
    o_b  = sum_j exp(-norm)                      # [N,H,W,B]
    out  = concat([x, o_b], axis=3)              # [N,H,W,F+B]

Sharding: data-parallel over the outer batch axis N (4 rows per core, 8
cores); every core receives the full x (as a pre-transposed fp16 copy) and
computes the whole M on-chip, then only its 4 rows of the pairwise block.

Per-core device layout ("L2"): M2_q [(b16,c8) partitions, (n32,hw256) free]
for each b-quarter q, so that
  - M-compute is a plain matmul (lhsT = T-tile [f,(b,c)], rhs = xT [f,(n,hw)])
  - the pairwise |M_j - M_i| is a DVE tensor op between free-dim slices
    (j-block read vs broadcast i-slice)
  - the c-reduction contracts the partition axis on the TensorEngine with
    stripe-ones matrices, accumulating 8 (q',i) stripes into one PSUM tile
    [(q',i,b) partitions, (j,hw) free]
  - exp(-norm) is one ACT pass, and the j-sum is a windowed DVE reduce
    over the strided j axis.
"""

import os
import sys

for _p in ("/opt/trn_rl_repo", "/opt/pypackages"):
    if _p not in sys.path and os.path.isdir(_p):
        sys.path.append(_p)

import numpy as np

N, HW, F, B, C = 32, 256, 256, 64, 8
NL = 4          # local rows per core
CORES = 8
FH = 2          # f in two partition halves of 128
Q = 4           # b-quarters of 16
HWC = 4         # hw chunks of 64
HW_CH = HW // HWC

F16 = "float16"


def _get_absdiff_op():
    """Fused |a-b| as a custom DVE op (one pass instead of sub+abs)."""
    if "absdiff" in _CACHED:
        return _CACHED["absdiff"]
    from concourse import dve_ops
    from concourse.dve_spec import Spec, Src0, Src1, lower, maxx
    from concourse.dve_uop import DveOpSpec

    for op in dve_ops.OPS:
        if op.name == "ABSDIFF_ANT":
            _CACHED["absdiff"] = op
            return op
    spec = Spec(
        body=maxx(Src0 - Src1, Src1 - Src0),
        reference=lambda in0, in1, s0, s1, imm2: np.abs(
            in0.astype(np.float32) - in1.astype(np.float32)
        ),
    )
    shas = {}
    for ver in ("v3", "v4"):
        shas[ver] = DveOpSpec(
            name="ABSDIFF_ANT", opcode=1, uops=lower(spec, ver=ver), rd1_en=True
        ).sha(ver)
    op = dve_ops.DveOp("ABSDIFF_ANT", spec, subdim=False, uops_sha=shas)
    dve_ops.OPS.append(op)
    dve_ops.CUSTOM_DVE_SPECS[op.name] = op.spec
    dve_ops._SUB_OPCODE_FOR_NAME[op.name] = (
        dve_ops._CUSTOM_DVE_ROW_BASE + len(dve_ops.OPS) - 1
    )
    _CACHED["absdiff"] = op
    return op


# --------------------------------------------------------------------------
# device program
# --------------------------------------------------------------------------

def build_body(tc, outs, ins):
    """Trace the per-core Tile program.

    ins:  xT   [2,128,8192] f16   xT[fh,f,n*256+hw] = x[n,hw,fh*128+f]
          xiT  [2,128,1024] f16   same, restricted to this core's 4 rows
          tw   [2,4,128,128] f16  tw[fh,q,f,b*8+c] = T[fh*128+f,16q+b,c]
          ones [8,128,128]  f16   ones[s,b*8+c,col] = (col == 16s+b)
    outs: o    [2,128,256]  f32   o[t, 64q'+16i+b, hw] = o_b[ib+i, hw, 16(2t+q')+b]
    """
    from contextlib import ExitStack

    import concourse.bass as bass
    import concourse.mybir as mybir

    nc = tc.nc
    f16 = mybir.dt.float16
    f32 = mybir.dt.float32

    xT_d, xiT_d, tw_d, ones_d = ins["xT"], ins["xiT"], ins["tw"], ins["ones"]
    o_d = outs["o"]

    with ExitStack() as ctx:
        singles = ctx.enter_context(tc.tile_pool(name="singles", bufs=1))
        psA = ctx.enter_context(tc.tile_pool(name="psA", bufs=2, space="PSUM"))
        psN = ctx.enter_context(tc.tile_pool(name="psN", bufs=2, space="PSUM"))
        adp = ctx.enter_context(tc.tile_pool(name="adp", bufs=10))
        Ep = ctx.enter_context(tc.tile_pool(name="Ep", bufs=2))

        # ---- loads -------------------------------------------------------
        xT_s, xiT_s, tw_s = [], [], []
        for fh in range(FH):
            t = singles.tile([128, N * HW], f16, tag=f"xT{fh}")
            nc.sync.dma_start(out=t, in_=xT_d[fh])
            xT_s.append(t)
            t = singles.tile([128, NL * HW], f16, tag=f"xiT{fh}")
            nc.sync.dma_start(out=t, in_=xiT_d[fh])
            xiT_s.append(t)
            row = []
            for q in range(Q):
                t = singles.tile([128, 128], f16, tag=f"tw{fh}{q}")
                nc.sync.dma_start(out=t, in_=tw_d[fh, q])
                row.append(t)
            tw_s.append(row)
        ones_s = []
        for s in range(8):
            t = singles.tile([128, 128], f16, tag=f"ones{s}")
            nc.sync.dma_start(out=t, in_=ones_d[s])
            ones_s.append(t)

        # ---- stage B: M2 = T' @ xT  (and M2i from the local rows) --------
        M2, M2i = [], []
        for q in range(Q):
            m2 = singles.tile([128, N * HW], f16, tag=f"m2{q}")
            for piece in range(8):          # 1024-col pieces
                ps = psA.tile([128, 1024], f32, tag="psA")
                for sub in range(2):        # 512-col matmuls
                    sl = slice(sub * 512, (sub + 1) * 512)
                    src = slice(piece * 1024 + sub * 512, piece * 1024 + (sub + 1) * 512)
                    for fh in range(FH):
                        nc.tensor.matmul(
                            ps[:, sl], lhsT=tw_s[fh][q], rhs=xT_s[fh][:, src],
                            start=(fh == 0), stop=(fh == 1),
                        )
                nc.scalar.copy(out=m2[:, piece * 1024:(piece + 1) * 1024], in_=ps[:])
            M2.append(m2)
        for q in range(Q):
            m2i = singles.tile([128, NL * HW], f16, tag=f"m2i{q}")
            ps = psA.tile([128, 1024], f32, tag="psA")
            for sub in range(2):
                sl = slice(sub * 512, (sub + 1) * 512)
                for fh in range(FH):
                    nc.tensor.matmul(
                        ps[:, sl], lhsT=tw_s[fh][q], rhs=xiT_s[fh][:, sl],
                        start=(fh == 0), stop=(fh == 1),
                    )
            nc.scalar.copy(out=m2i, in_=ps[:])
            M2i.append(m2i)

        # ---- stage C: pairwise |diff|, c-reduce, exp, j-sum --------------
        for t in range(2):                  # b-quarter pairs (q = 2t+q')
            o_sb = singles.tile([128, HW], f32, tag=f"osb{t}")
            for hwc in range(HWC):
                hsl = slice(hwc * HW_CH, (hwc + 1) * HW_CH)
                ads = []
                for qp in range(2):
                    q = 2 * t + qp
                    m2v = M2[q].rearrange("p (n hw) -> p n hw", n=N)
                    m2iv = M2i[q].rearrange("p (i hw) -> p i hw", i=NL)
                    for i in range(NL):
                        ad = adp.tile([128, N * HW_CH], f16, tag="ad")
                        adv = ad.rearrange("p (n hw) -> p n hw", n=N)
                        src0 = m2v[:, :, hsl]
                        s1 = m2iv[:, i, hsl]           # [128, 64]
                        src1 = bass.AP(
                            tensor=s1.tensor, offset=s1.offset,
                            ap=[list(s1.ap[0]), [0, N], list(s1.ap[1])],
                        )
                        nc.vector._custom_dve(
                            _get_absdiff_op(), out=adv, in0=src0, in1=src1,
                        )
                        ads.append(ad)
                E = Ep.tile([128, N * HW_CH], f32, tag="E")
                for h in range(2):          # two 1024-col norm tiles
                    nrm = psN.tile([128, 1024], f32, tag="nrm")
                    for sub in range(2):
                        sl = slice(sub * 512, (sub + 1) * 512)
                        cc = slice(h * 1024 + sub * 512, h * 1024 + (sub + 1) * 512)
                        for s in range(8):
                            nc.tensor.matmul(
                                nrm[:, sl], lhsT=ones_s[s], rhs=ads[s][:, cc],
                                start=(s == 0), stop=(s == 7),
                            )
                    nc.scalar.activation(
                        out=E[:, h * 1024:(h + 1) * 1024], in_=nrm[:],
                        func=mybir.ActivationFunctionType.Exp, scale=-1.0,
                    )
                Ev = E.rearrange("p (j hw) -> p hw j", j=N)
                nc.vector.tensor_reduce(
                    out=o_sb[:, hsl], in_=Ev,
                    axis=mybir.AxisListType.X, op=mybir.AluOpType.add,
                )
            nc.sync.dma_start(out=o_d[t], in_=o_sb)


# --------------------------------------------------------------------------
# host side
# --------------------------------------------------------------------------

def prep_inputs(x, T):
    """Shared (core-independent) device inputs."""
    xf = np.ascontiguousarray(x.reshape(N, HW, F))
    xT_np = np.ascontiguousarray(xf.transpose(2, 0, 1).reshape(F, N * HW))
    xT_in = xT_np.reshape(FH, 128, N * HW).astype(np.float16)
    tw = T.reshape(FH, 128, Q, 16, C).transpose(0, 2, 1, 3, 4)
    tw_in = np.ascontiguousarray(tw.reshape(FH, Q, 128, 128)).astype(np.float16)
    ones_in = np.zeros((8, 128, 128), np.float16)
    for s in range(8):
        for b in range(16):
            ones_in[s, b * 8:(b + 1) * 8, 16 * s + b] = 1.0
    return xT_np, xT_in, tw_in, ones_in


def core_in_map(xT_np, xT_in, tw_in, ones_in, k):
    xiT = np.ascontiguousarray(
        xT_np[:, k * NL * HW:(k + 1) * NL * HW]
    ).reshape(FH, 128, NL * HW).astype(np.float16)
    return {"xT": xT_in, "xiT": xiT, "tw": tw_in, "ones": ones_in}


def gather_ob(core_outs):
    """core_outs: list of 8 arrays [2,128,256] f32 -> o_b [N,16,16,B]."""
    obs = []
    for res in core_outs:
        v = res.reshape(2, 2, NL, 16, HW)          # t, q', i, b, hw
        obs.append(v.transpose(2, 4, 0, 1, 3).reshape(NL, HW, B))
    return np.concatenate(obs, axis=0).reshape(N, 16, 16, B)


_CACHED = {}


def _get_program():
    if "nc" in _CACHED:
        return _CACHED["nc"]
    from contextlib import ExitStack

    import concourse.bacc as bacc
    import concourse.mybir as mybir
    import concourse.tile as tile

    nc = bacc.Bacc("TRN2", target_bir_lowering=False, debug=False,
                   num_devices=CORES)
    f16, f32 = mybir.dt.float16, mybir.dt.float32
    ins = {
        "xT": nc.dram_tensor("xT", [FH, 128, N * HW], f16, kind="ExternalInput").ap(),
        "xiT": nc.dram_tensor("xiT", [FH, 128, NL * HW], f16, kind="ExternalInput").ap(),
        "tw": nc.dram_tensor("tw", [FH, Q, 128, 128], f16, kind="ExternalInput").ap(),
        "ones": nc.dram_tensor("ones", [8, 128, 128], f16, kind="ExternalInput").ap(),
    }
    outs = {
        "o": nc.dram_tensor("o", [2, 128, HW], f32, kind="ExternalOutput").ap(),
    }
    with tile.TileContext(nc) as tc:
        build_body(tc, outs, ins)
    nc.compile()
    _CACHED["nc"] = nc
    return nc


def kernel(x, T):
    x = np.asarray(x, dtype=np.float32)
    T = np.asarray(T, dtype=np.float32)
    from concourse.bass_utils import run_bass_kernel_spmd

    nc = _get_program()
    xT_np, xT_in, tw_in, ones_in = prep_inputs(x, T)
    in_maps = [core_in_map(xT_np, xT_in, tw_in, ones_in, k) for k in range(CORES)]
    res = run_bass_kernel_spmd(nc, in_maps, core_ids=list(range(CORES)))
    ob = gather_ob([r["o"] for r in res.results])
    return np.concatenate([x, ob], axis=3)


# revision 6
# speedup vs baseline: 1.1820x; 1.1604x over previous
"""MiniBatchDiscrimination Trainium2 kernel.

reference:
    M = einsum('nhwf,fbc->nhwbc', x, T)          # [N,H,W,B,C]
    norm = sum_c |M[i] - M[j]|                   # [N,N,H,W,B]
    o_b  = sum_j exp(-norm)                      # [N,H,W,B]
    out  = concat([x, o_b], axis=3)              # [N,H,W,F+B]

Sharding: data-parallel over the outer batch axis N (4 rows per core, 8
cores); every core receives the full x (as a pre-transposed fp16 copy) and
computes the whole M on-chip, then only its 4 rows of the pairwise block.

Per-core device layout ("L2"): M2_q [(b16,c8) partitions, (n32,hw256) free]
for each b-quarter q, so that
  - M-compute is a plain matmul (lhsT = T-tile [f,(b,c)], rhs = xT [f,(n,hw)])
  - the pairwise |M_j - M_i| is a DVE tensor op between free-dim slices
    (j-block read vs broadcast i-slice)
  - the c-reduction contracts the partition axis on the TensorEngine with
    stripe-ones matrices, accumulating 8 (q',i) stripes into one PSUM tile
    [(q',i,b) partitions, (j,hw) free]
  - exp(-norm) is one ACT pass, and the j-sum is a windowed DVE reduce
    over the strided j axis.
"""

import os
import sys

for _p in ("/opt/trn_rl_repo", "/opt/pypackages"):
    if _p not in sys.path and os.path.isdir(_p):
        sys.path.append(_p)

import numpy as np

N, HW, F, B, C = 32, 256, 256, 64, 8
NL = 4          # local rows per core
CORES = 8
FH = 2          # f in two partition halves of 128
Q = 4           # b-quarters of 16
HWC = 4         # hw chunks of 64
HW_CH = HW // HWC

F16 = "float16"


def _absdiff_uop_1x():
    """REGULAR program: |a-b| via SUB, reverse-SUB, MAX on slices 0-2."""
    from concourse.dve_uop import (
        ENABLE, AluInp, AluOp, DelayInp, InpSel, OutPath, OutSel, Trigger,
        UopConfig, UopDpConfig,
    )

    u = UopConfig()
    u.enable_input(InpSel.SRC_0, 0).enable_input(InpSel.SRC_1, 1)
    u.require_inp0 = ENABLE
    u.require_inp1 = ENABLE
    u.trigger = (Trigger.SRC_TENSOR_DONE, Trigger.NONE, Trigger.NONE)
    u.enable_output(OutSel.ALU_OUT, OutPath.WR0_LO)
    dp = u.datapath_config
    # s0: alu = a - b; carry b (chain0), capture a (chain3)
    dp[0] = (UopDpConfig()
             .enable_alu(AluOp.SUBTRACT, AluInp.PREV_ALU_OUT, AluInp.PREV_DELAY_0)
             .pass_through_delay(0)
             .enable_delay_from_src(DelayInp.PREV_ALU_OUT, 3))
    # s1: alu = b - a; capture (a-b) into chain0
    dp[1] = (UopDpConfig()
             .enable_alu(AluOp.SUBTRACT, AluInp.PREV_DELAY_0, AluInp.PREV_DELAY_3)
             .enable_delay_from_src(DelayInp.PREV_ALU_OUT, 0))
    # s2: alu = max(b-a, a-b)
    dp[2] = UopDpConfig().enable_alu(
        AluOp.MAX, AluInp.PREV_ALU_OUT, AluInp.PREV_DELAY_0)
    for i in range(3, 8):
        dp[i] = UopDpConfig().pass_through_alu()
    return u


def _absdiff_uop_2x():
    """2X_1PORT program: lo on slices 0-2, hi on slices 3-5."""
    from concourse.dve_uop import (
        ENABLE, AluInp, AluOp, DelayInp, InpSel, OutPath, OutSel, Trigger,
        UopConfig, UopDpConfig,
    )

    u = UopConfig()
    u.enable_input(InpSel.SRC_0, 0).enable_input(InpSel.SRC_1, 1)
    u.enable_input(InpSel.SRC_0_HI, 2).enable_input(InpSel.SRC_1_HI, 3)
    u.require_inp0 = ENABLE
    u.require_inp1 = ENABLE
    u.trigger = (Trigger.SRC_TENSOR_DONE, Trigger.NONE, Trigger.NONE)
    u.enable_output(OutSel.DELAY_0, OutPath.WR0_LO)   # lo result rides chain0
    u.enable_output(OutSel.ALU_OUT, OutPath.WR0_HI)   # hi result on ALU lane
    dp = u.datapath_config
    # s0: alu = a_lo - b_lo; carry b_lo(c0), a_hi(c1), b_hi(c2); capture a_lo(c3)
    dp[0] = (UopDpConfig()
             .enable_alu(AluOp.SUBTRACT, AluInp.PREV_ALU_OUT, AluInp.PREV_DELAY_0)
             .pass_through_delay(0, 1, 2)
             .enable_delay_from_src(DelayInp.PREV_ALU_OUT, 3))
    # s1: alu = b_lo - a_lo; capture (a-b)_lo into c0; carry a_hi, b_hi
    dp[1] = (UopDpConfig()
             .enable_alu(AluOp.SUBTRACT, AluInp.PREV_DELAY_0, AluInp.PREV_DELAY_3)
             .enable_delay_from_src(DelayInp.PREV_ALU_OUT, 0)
             .pass_through_delay(1, 2))
    # s2: alu = max -> |a-b|_lo; carry a_hi, b_hi
    dp[2] = (UopDpConfig()
             .enable_alu(AluOp.MAX, AluInp.PREV_ALU_OUT, AluInp.PREV_DELAY_0)
             .pass_through_delay(1, 2))
    # s3: alu = a_hi - b_hi; capture lo result into c0; carry a_hi, b_hi
    dp[3] = (UopDpConfig()
             .enable_alu(AluOp.SUBTRACT, AluInp.PREV_DELAY_1, AluInp.PREV_DELAY_2)
             .enable_delay_from_src(DelayInp.PREV_ALU_OUT, 0)
             .pass_through_delay(1, 2))
    # s4: alu = b_hi - a_hi; carry lo(c0); capture (a-b)_hi into c3
    dp[4] = (UopDpConfig()
             .enable_alu(AluOp.SUBTRACT, AluInp.PREV_DELAY_2, AluInp.PREV_DELAY_1)
             .pass_through_delay(0)
             .enable_delay_from_src(DelayInp.PREV_ALU_OUT, 3))
    # s5: alu = max -> |a-b|_hi; carry lo(c0)
    dp[5] = (UopDpConfig()
             .enable_alu(AluOp.MAX, AluInp.PREV_ALU_OUT, AluInp.PREV_DELAY_3)
             .pass_through_delay(0))
    # s6, s7: pass alu (hi) + chain0 (lo)
    for i in (6, 7):
        dp[i] = UopDpConfig().pass_through_alu().pass_through_delay(0)
    return u


def _get_absdiff_op():
    """Fused |a-b| custom DVE op with a hand-written 2X_1PORT variant."""
    if "absdiff" in _CACHED:
        return _CACHED["absdiff"]
    from concourse import dve_ops
    from concourse.dve_spec import Spec, Src0, Src1, maxx
    from concourse.dve_uop import DveOpSpec

    NAME = "ABSDIFF_ANT"
    for op in dve_ops.OPS:
        if op.name == NAME:
            _CACHED["absdiff"] = op
            return op
    spec = Spec(
        body=maxx(Src0 - Src1, Src1 - Src0),
        reference=lambda in0, in1, s0, s1, imm2: np.abs(
            in0.astype(np.float32) - in1.astype(np.float32)
        ),
    )
    op = dve_ops.DveOp(NAME, spec, subdim=False, uops_sha={})
    dve_ops.OPS.append(op)
    dve_ops.CUSTOM_DVE_SPECS[op.name] = op.spec
    row = dve_ops._CUSTOM_DVE_ROW_BASE + len(dve_ops.OPS) - 1
    dve_ops._SUB_OPCODE_FOR_NAME[op.name] = row
    compiled = DveOpSpec(
        name=NAME,
        opcode=row,
        uops=[_absdiff_uop_1x()],
        uops_2x=[_absdiff_uop_2x()],
        perf_max=1,
        rd1_en=True,
    )
    compiled.validate("v3")
    dve_ops._COMPILE_CACHE[(NAME, "v3")] = compiled
    dve_ops._COMPILE_CACHE[(NAME, "v4")] = compiled
    _CACHED["absdiff"] = op
    return op


# --------------------------------------------------------------------------
# device program
# --------------------------------------------------------------------------

def build_body(tc, outs, ins):
    """Trace the per-core Tile program.

    ins:  xT   [2,128,8192] f16   xT[fh,f,n*256+hw] = x[n,hw,fh*128+f]
          xiT  [2,128,1024] f16   same, restricted to this core's 4 rows
          tw   [2,4,128,128] f16  tw[fh,q,f,b*8+c] = T[fh*128+f,16q+b,c]
          ones [8,128,128]  f16   ones[s,b*8+c,col] = (col == 16s+b)
    outs: o    [2,128,256]  f32   o[t, 64q'+16i+b, hw] = o_b[ib+i, hw, 16(2t+q')+b]
    """
    from contextlib import ExitStack

    import concourse.bass as bass
    import concourse.mybir as mybir

    nc = tc.nc
    f16 = mybir.dt.float16
    f32 = mybir.dt.float32

    xT_d, xiT_d, tw_d, ones_d = ins["xT"], ins["xiT"], ins["tw"], ins["ones"]
    o_d = outs["o"]

    with ExitStack() as ctx:
        singles = ctx.enter_context(tc.tile_pool(name="singles", bufs=1))
        psA = ctx.enter_context(tc.tile_pool(name="psA", bufs=2, space="PSUM"))
        psN = ctx.enter_context(tc.tile_pool(name="psN", bufs=2, space="PSUM"))
        adp = ctx.enter_context(tc.tile_pool(name="adp", bufs=10))
        Ep = ctx.enter_context(tc.tile_pool(name="Ep", bufs=2))

        # ---- loads -------------------------------------------------------
        xT_s, xiT_s, tw_s = [], [], []
        for fh in range(FH):
            t = singles.tile([128, N * HW], f16, tag=f"xT{fh}")
            nc.sync.dma_start(out=t, in_=xT_d[fh])
            xT_s.append(t)
            t = singles.tile([128, NL * HW], f16, tag=f"xiT{fh}")
            nc.sync.dma_start(out=t, in_=xiT_d[fh])
            xiT_s.append(t)
            row = []
            for q in range(Q):
                t = singles.tile([128, 128], f16, tag=f"tw{fh}{q}")
                nc.sync.dma_start(out=t, in_=tw_d[fh, q])
                row.append(t)
            tw_s.append(row)
        ones_s = []
        for s in range(8):
            t = singles.tile([128, 128], f16, tag=f"ones{s}")
            nc.sync.dma_start(out=t, in_=ones_d[s])
            ones_s.append(t)

        # ---- stage B: M2 = T' @ xT  (and M2i from the local rows) --------
        M2, M2i = [], []
        for q in range(Q):
            m2 = singles.tile([128, N * HW], f16, tag=f"m2{q}")
            for piece in range(8):          # 1024-col pieces
                ps = psA.tile([128, 1024], f32, tag="psA")
                for sub in range(2):        # 512-col matmuls
                    sl = slice(sub * 512, (sub + 1) * 512)
                    src = slice(piece * 1024 + sub * 512, piece * 1024 + (sub + 1) * 512)
                    for fh in range(FH):
                        nc.tensor.matmul(
                            ps[:, sl], lhsT=tw_s[fh][q], rhs=xT_s[fh][:, src],
                            start=(fh == 0), stop=(fh == 1),
                        )
                nc.scalar.copy(out=m2[:, piece * 1024:(piece + 1) * 1024], in_=ps[:])
            M2.append(m2)
        for q in range(Q):
            m2i = singles.tile([128, NL * HW], f16, tag=f"m2i{q}")
            ps = psA.tile([128, 1024], f32, tag="psA")
            for sub in range(2):
                sl = slice(sub * 512, (sub + 1) * 512)
                for fh in range(FH):
                    nc.tensor.matmul(
                        ps[:, sl], lhsT=tw_s[fh][q], rhs=xiT_s[fh][:, sl],
                        start=(fh == 0), stop=(fh == 1),
                    )
            nc.scalar.copy(out=m2i, in_=ps[:])
            M2i.append(m2i)

        # ---- stage C: pairwise |diff|, c-reduce, exp, j-sum --------------
        for t in range(2):                  # b-quarter pairs (q = 2t+q')
            o_sb = singles.tile([128, HW], f32, tag=f"osb{t}")
            for hwc in range(HWC):
                hsl = slice(hwc * HW_CH, (hwc + 1) * HW_CH)
                ads = []
                for qp in range(2):
                    q = 2 * t + qp
                    m2v = M2[q].rearrange("p (n hw) -> p n hw", n=N)
                    m2iv = M2i[q].rearrange("p (i hw) -> p i hw", i=NL)
                    for i in range(NL):
                        ad = adp.tile([128, N * HW_CH], f16, tag="ad")
                        adv = ad.rearrange("p (n hw) -> p n hw", n=N)
                        src0 = m2v[:, :, hsl]
                        s1 = m2iv[:, i, hsl]           # [128, 64]
                        src1 = bass.AP(
                            tensor=s1.tensor, offset=s1.offset,
                            ap=[list(s1.ap[0]), [0, N], list(s1.ap[1])],
                        )
                        bi = nc.vector._custom_dve(
                            _get_absdiff_op(), out=adv, in0=src0, in1=src1,
                        )
                        bi.ins.perf_max = 1
                        ads.append(ad)
                E = Ep.tile([128, N * HW_CH], f32, tag="E")
                for h in range(2):          # two 1024-col norm tiles
                    nrm = psN.tile([128, 1024], f32, tag="nrm")
                    for sub in range(2):
                        sl = slice(sub * 512, (sub + 1) * 512)
                        cc = slice(h * 1024 + sub * 512, h * 1024 + (sub + 1) * 512)
                        for s in range(8):
                            nc.tensor.matmul(
                                nrm[:, sl], lhsT=ones_s[s], rhs=ads[s][:, cc],
                                start=(s == 0), stop=(s == 7),
                            )
                    nc.scalar.activation(
                        out=E[:, h * 1024:(h + 1) * 1024], in_=nrm[:],
                        func=mybir.ActivationFunctionType.Exp, scale=-1.0,
                    )
                Ev = E.rearrange("p (j hw) -> p hw j", j=N)
                nc.vector.tensor_reduce(
                    out=o_sb[:, hsl], in_=Ev,
                    axis=mybir.AxisListType.X, op=mybir.AluOpType.add,
                )
            nc.sync.dma_start(out=o_d[t], in_=o_sb)


# --------------------------------------------------------------------------
# host side
# --------------------------------------------------------------------------

def prep_inputs(x, T):
    """Shared (core-independent) device inputs."""
    xf = np.ascontiguousarray(x.reshape(N, HW, F))
    xT_np = np.ascontiguousarray(xf.transpose(2, 0, 1).reshape(F, N * HW))
    xT_in = xT_np.reshape(FH, 128, N * HW).astype(np.float16)
    tw = T.reshape(FH, 128, Q, 16, C).transpose(0, 2, 1, 3, 4)
    tw_in = np.ascontiguousarray(tw.reshape(FH, Q, 128, 128)).astype(np.float16)
    ones_in = np.zeros((8, 128, 128), np.float16)
    for s in range(8):
        for b in range(16):
            ones_in[s, b * 8:(b + 1) * 8, 16 * s + b] = 1.0
    return xT_np, xT_in, tw_in, ones_in


def core_in_map(xT_np, xT_in, tw_in, ones_in, k):
    xiT = np.ascontiguousarray(
        xT_np[:, k * NL * HW:(k + 1) * NL * HW]
    ).reshape(FH, 128, NL * HW).astype(np.float16)
    return {"xT": xT_in, "xiT": xiT, "tw": tw_in, "ones": ones_in}


def gather_ob(core_outs):
    """core_outs: list of 8 arrays [2,128,256] f32 -> o_b [N,16,16,B]."""
    obs = []
    for res in core_outs:
        v = res.reshape(2, 2, NL, 16, HW)          # t, q', i, b, hw
        obs.append(v.transpose(2, 4, 0, 1, 3).reshape(NL, HW, B))
    return np.concatenate(obs, axis=0).reshape(N, 16, 16, B)


_CACHED = {}


def _get_program():
    if "nc" in _CACHED:
        return _CACHED["nc"]
    from contextlib import ExitStack

    import concourse.bacc as bacc
    import concourse.mybir as mybir
    import concourse.tile as tile

    nc = bacc.Bacc("TRN2", target_bir_lowering=False, debug=False,
                   num_devices=CORES)
    f16, f32 = mybir.dt.float16, mybir.dt.float32
    ins = {
        "xT": nc.dram_tensor("xT", [FH, 128, N * HW], f16, kind="ExternalInput").ap(),
        "xiT": nc.dram_tensor("xiT", [FH, 128, NL * HW], f16, kind="ExternalInput").ap(),
        "tw": nc.dram_tensor("tw", [FH, Q, 128, 128], f16, kind="ExternalInput").ap(),
        "ones": nc.dram_tensor("ones", [8, 128, 128], f16, kind="ExternalInput").ap(),
    }
    outs = {
        "o": nc.dram_tensor("o", [2, 128, HW], f32, kind="ExternalOutput").ap(),
    }
    with tile.TileContext(nc) as tc:
        build_body(tc, outs, ins)
    nc.compile()
    _CACHED["nc"] = nc
    return nc


def kernel(x, T):
    x = np.asarray(x, dtype=np.float32)
    T = np.asarray(T, dtype=np.float32)
    from concourse.bass_utils import run_bass_kernel_spmd

    nc = _get_program()
    xT_np, xT_in, tw_in, ones_in = prep_inputs(x, T)
    in_maps = [core_in_map(xT_np, xT_in, tw_in, ones_in, k) for k in range(CORES)]
    res = run_bass_kernel_spmd(nc, in_maps, core_ids=list(range(CORES)))
    ob = gather_ob([r["o"] for r in res.results])
    return np.concatenate([x, ob], axis=3)


# revision 9
# speedup vs baseline: 9.7926x; 8.2850x over previous
"""MiniBatchDiscrimination Trainium2 kernel.

reference:
    M = einsum('nhwf,fbc->nhwbc', x, T)          # [N,H,W,B,C]
    norm = sum_c |M[i] - M[j]|                   # [N,N,H,W,B]
    o_b  = sum_j exp(-norm)                      # [N,H,W,B]
    out  = concat([x, o_b], axis=3)              # [N,H,W,F+B]

Sharding: data-parallel over the outer batch axis N (4 rows per core, 8
cores); every core receives the full x (as a pre-transposed fp16 copy) and
computes the whole M on-chip, then only its 4 rows of the pairwise block.

Per-core device layout ("L2"): M2_q [(b16,c8) partitions, (n32,hw256) free]
for each b-quarter q, so that
  - M-compute is a plain matmul (lhsT = T-tile [f,(b,c)], rhs = xT [f,(n,hw)])
  - the pairwise |M_j - M_i| is a DVE tensor op between free-dim slices
    (j-block read vs broadcast i-slice)
  - the c-reduction contracts the partition axis on the TensorEngine with
    stripe-ones matrices, accumulating 8 (q',i) stripes into one PSUM tile
    [(q',i,b) partitions, (j,hw) free]
  - exp(-norm) is one ACT pass, and the j-sum is a windowed DVE reduce
    over the strided j axis.
"""

import os
import sys

for _p in ("/opt/trn_rl_repo", "/opt/pypackages"):
    if _p not in sys.path and os.path.isdir(_p):
        sys.path.append(_p)

import numpy as np

N, HW, F, B, C = 32, 256, 256, 64, 8
NL = 4          # local rows per core
CORES = 8
FH = 2          # f in two partition halves of 128
Q = 4           # b-quarters of 16
HWC = 4         # hw chunks of 64
HW_CH = HW // HWC

F16 = "float16"


def _absdiff_uop_1x():
    """REGULAR program: |a-b| via SUB, reverse-SUB, MAX on slices 0-2."""
    from concourse.dve_uop import (
        ENABLE, AluInp, AluOp, DelayInp, InpSel, OutPath, OutSel, Trigger,
        UopConfig, UopDpConfig,
    )

    u = UopConfig()
    u.enable_input(InpSel.SRC_0, 0).enable_input(InpSel.SRC_1, 1)
    u.require_inp0 = ENABLE
    u.require_inp1 = ENABLE
    u.trigger = (Trigger.SRC_TENSOR_DONE, Trigger.NONE, Trigger.NONE)
    u.enable_output(OutSel.ALU_OUT, OutPath.WR0_LO)
    dp = u.datapath_config
    # s0: alu = a - b; carry b (chain0), capture a (chain3)
    dp[0] = (UopDpConfig()
             .enable_alu(AluOp.SUBTRACT, AluInp.PREV_ALU_OUT, AluInp.PREV_DELAY_0)
             .pass_through_delay(0)
             .enable_delay_from_src(DelayInp.PREV_ALU_OUT, 3))
    # s1: alu = b - a; capture (a-b) into chain0
    dp[1] = (UopDpConfig()
             .enable_alu(AluOp.SUBTRACT, AluInp.PREV_DELAY_0, AluInp.PREV_DELAY_3)
             .enable_delay_from_src(DelayInp.PREV_ALU_OUT, 0))
    # s2: alu = max(b-a, a-b)
    dp[2] = UopDpConfig().enable_alu(
        AluOp.MAX, AluInp.PREV_ALU_OUT, AluInp.PREV_DELAY_0)
    for i in range(3, 8):
        dp[i] = UopDpConfig().pass_through_alu()
    return u


def _absdiff_uop_2x():
    """2X_1PORT program: lo on slices 0-2, hi on slices 3-5."""
    from concourse.dve_uop import (
        ENABLE, AluInp, AluOp, DelayInp, InpSel, OutPath, OutSel, Trigger,
        UopConfig, UopDpConfig,
    )

    u = UopConfig()
    u.enable_input(InpSel.SRC_0, 0).enable_input(InpSel.SRC_1, 1)
    u.enable_input(InpSel.SRC_0_HI, 2).enable_input(InpSel.SRC_1_HI, 3)
    u.require_inp0 = ENABLE
    u.require_inp1 = ENABLE
    u.trigger = (Trigger.SRC_TENSOR_DONE, Trigger.NONE, Trigger.NONE)
    u.enable_output(OutSel.DELAY_0, OutPath.WR0_LO)   # lo result rides chain0
    u.enable_output(OutSel.ALU_OUT, OutPath.WR0_HI)   # hi result on ALU lane
    dp = u.datapath_config
    # s0: alu = a_lo - b_lo; carry b_lo(c0), a_hi(c1), b_hi(c2); capture a_lo(c3)
    dp[0] = (UopDpConfig()
             .enable_alu(AluOp.SUBTRACT, AluInp.PREV_ALU_OUT, AluInp.PREV_DELAY_0)
             .pass_through_delay(0, 1, 2)
             .enable_delay_from_src(DelayInp.PREV_ALU_OUT, 3))
    # s1: alu = b_lo - a_lo; capture (a-b)_lo into c0; carry a_hi, b_hi
    dp[1] = (UopDpConfig()
             .enable_alu(AluOp.SUBTRACT, AluInp.PREV_DELAY_0, AluInp.PREV_DELAY_3)
             .enable_delay_from_src(DelayInp.PREV_ALU_OUT, 0)
             .pass_through_delay(1, 2))
    # s2: alu = max -> |a-b|_lo; carry a_hi, b_hi
    dp[2] = (UopDpConfig()
             .enable_alu(AluOp.MAX, AluInp.PREV_ALU_OUT, AluInp.PREV_DELAY_0)
             .pass_through_delay(1, 2))
    # s3: alu = a_hi - b_hi; capture lo result into c0; carry a_hi, b_hi
    dp[3] = (UopDpConfig()
             .enable_alu(AluOp.SUBTRACT, AluInp.PREV_DELAY_1, AluInp.PREV_DELAY_2)
             .enable_delay_from_src(DelayInp.PREV_ALU_OUT, 0)
             .pass_through_delay(1, 2))
    # s4: alu = b_hi - a_hi; carry lo(c0); capture (a-b)_hi into c3
    dp[4] = (UopDpConfig()
             .enable_alu(AluOp.SUBTRACT, AluInp.PREV_DELAY_2, AluInp.PREV_DELAY_1)
             .pass_through_delay(0)
             .enable_delay_from_src(DelayInp.PREV_ALU_OUT, 3))
    # s5: alu = max -> |a-b|_hi; carry lo(c0)
    dp[5] = (UopDpConfig()
             .enable_alu(AluOp.MAX, AluInp.PREV_ALU_OUT, AluInp.PREV_DELAY_3)
             .pass_through_delay(0))
    # s6, s7: pass alu (hi) + chain0 (lo)
    for i in (6, 7):
        dp[i] = UopDpConfig().pass_through_alu().pass_through_delay(0)
    return u


def _get_absdiff_op():
    """Fused |a-b| custom DVE op with a hand-written 2X_1PORT variant."""
    if "absdiff" in _CACHED:
        return _CACHED["absdiff"]
    from concourse import dve_ops
    from concourse.dve_spec import Spec, Src0, Src1, maxx
    from concourse.dve_uop import DveOpSpec

    NAME = "ABSDIFF_ANT"
    for op in dve_ops.OPS:
        if op.name == NAME:
            _CACHED["absdiff"] = op
            return op
    spec = Spec(
        body=maxx(Src0 - Src1, Src1 - Src0),
        reference=lambda in0, in1, s0, s1, imm2: np.abs(
            in0.astype(np.float32) - in1.astype(np.float32)
        ),
    )
    op = dve_ops.DveOp(NAME, spec, subdim=False, uops_sha={})
    dve_ops.OPS.append(op)
    dve_ops.CUSTOM_DVE_SPECS[op.name] = op.spec
    row = dve_ops._CUSTOM_DVE_ROW_BASE + len(dve_ops.OPS) - 1
    dve_ops._SUB_OPCODE_FOR_NAME[op.name] = row
    compiled = DveOpSpec(
        name=NAME,
        opcode=row,
        uops=[_absdiff_uop_1x()],
        uops_2x=[_absdiff_uop_2x()],
        perf_max=1,
        rd1_en=True,
    )
    compiled.validate("v3")
    dve_ops._COMPILE_CACHE[(NAME, "v3")] = compiled
    dve_ops._COMPILE_CACHE[(NAME, "v4")] = compiled
    _CACHED["absdiff"] = op
    return op


# --------------------------------------------------------------------------
# device program
# --------------------------------------------------------------------------

def build_body(tc, outs, ins, rep=0):
    """Trace the per-core Tile program.

    ins:  xT   [2,128,8192] f16   xT[fh,f,n*256+hw] = x[n,hw,fh*128+f]
          xiT  [2,128,1024] f16   same, restricted to this core's 4 rows
          tw   [2,4,128,128] f16  tw[fh,q,f,b*8+c] = T[fh*128+f,16q+b,c]
          ones [8,128,128]  f16   ones[s,b*8+c,col] = (col == 16s+b)
    outs: o    [2,128,256]  f32   o[t, 64q'+16i+b, hw] = o_b[ib+i, hw, 16(2t+q')+b]
    """
    from contextlib import ExitStack

    import concourse.bass as bass
    import concourse.mybir as mybir

    nc = tc.nc
    f16 = mybir.dt.float16
    f32 = mybir.dt.float32

    xT_d, xiT_d, tw_d, ones_d = ins["xT"], ins["xiT"], ins["tw"], ins["ones"]
    o_d = outs["o"]

    with ExitStack() as ctx:
        sfx = f"_{rep}"
        singles = ctx.enter_context(tc.tile_pool(name="singles" + sfx, bufs=1))
        psA = ctx.enter_context(tc.tile_pool(name="psA" + sfx, bufs=2, space="PSUM"))
        psN = ctx.enter_context(tc.tile_pool(name="psN" + sfx, bufs=2, space="PSUM"))
        adp = ctx.enter_context(tc.tile_pool(name="adp" + sfx, bufs=10))
        Ep = ctx.enter_context(tc.tile_pool(name="Ep" + sfx, bufs=2))

        # ---- loads -------------------------------------------------------
        xT_s, xiT_s, tw_s = [], [], []
        for fh in range(FH):
            t = singles.tile([128, N * HW], f16, tag=f"xT{fh}")
            nc.sync.dma_start(out=t, in_=xT_d[fh])
            xT_s.append(t)
            t = singles.tile([128, NL * HW], f16, tag=f"xiT{fh}")
            nc.sync.dma_start(out=t, in_=xiT_d[fh])
            xiT_s.append(t)
            row = []
            for q in range(Q):
                t = singles.tile([128, 128], f16, tag=f"tw{fh}{q}")
                nc.sync.dma_start(out=t, in_=tw_d[fh, q])
                row.append(t)
            tw_s.append(row)
        ones_s = []
        for s in range(8):
            t = singles.tile([128, 128], f16, tag=f"ones{s}")
            nc.sync.dma_start(out=t, in_=ones_d[s])
            ones_s.append(t)

        # ---- stage B: M2 = T' @ xT  (and M2i from the local rows) --------
        M2, M2i = [], []
        for q in range(Q):
            m2 = singles.tile([128, N * HW], f16, tag=f"m2{q}")
            for piece in range(8):          # 1024-col pieces
                ps = psA.tile([128, 1024], f32, tag="psA")
                for sub in range(2):        # 512-col matmuls
                    sl = slice(sub * 512, (sub + 1) * 512)
                    src = slice(piece * 1024 + sub * 512, piece * 1024 + (sub + 1) * 512)
                    for fh in range(FH):
                        nc.tensor.matmul(
                            ps[:, sl], lhsT=tw_s[fh][q], rhs=xT_s[fh][:, src],
                            start=(fh == 0), stop=(fh == 1),
                        )
                nc.scalar.copy(out=m2[:, piece * 1024:(piece + 1) * 1024], in_=ps[:])
            M2.append(m2)
        for q in range(Q):
            m2i = singles.tile([128, NL * HW], f16, tag=f"m2i{q}")
            ps = psA.tile([128, 1024], f32, tag="psA")
            for sub in range(2):
                sl = slice(sub * 512, (sub + 1) * 512)
                for fh in range(FH):
                    nc.tensor.matmul(
                        ps[:, sl], lhsT=tw_s[fh][q], rhs=xiT_s[fh][:, sl],
                        start=(fh == 0), stop=(fh == 1),
                    )
            nc.scalar.copy(out=m2i, in_=ps[:])
            M2i.append(m2i)

        # ---- stage C: pairwise |diff|, c-reduce, exp, j-sum --------------
        for t in range(2):                  # b-quarter pairs (q = 2t+q')
            o_sb = singles.tile([128, HW], f32, tag=f"osb{t}")
            for hwc in range(HWC):
                hsl = slice(hwc * HW_CH, (hwc + 1) * HW_CH)
                ads = []
                for qp in range(2):
                    q = 2 * t + qp
                    m2v = M2[q].rearrange("p (n hw) -> p n hw", n=N)
                    m2iv = M2i[q].rearrange("p (i hw) -> p i hw", i=NL)
                    for i in range(NL):
                        ad = adp.tile([128, N * HW_CH], f16, tag="ad")
                        adv = ad.rearrange("p (n hw) -> p n hw", n=N)
                        src0 = m2v[:, :, hsl]
                        s1 = m2iv[:, i, hsl]           # [128, 64]
                        src1 = bass.AP(
                            tensor=s1.tensor, offset=s1.offset,
                            ap=[list(s1.ap[0]), [0, N], list(s1.ap[1])],
                        )
                        bi = nc.vector._custom_dve(
                            _get_absdiff_op(), out=adv, in0=src0, in1=src1,
                        )
                        bi.ins.perf_max = 1
                        ads.append(ad)
                E = Ep.tile([128, N * HW_CH], f32, tag="E")
                for h in range(2):          # two 1024-col norm tiles
                    nrm = psN.tile([128, 1024], f32, tag="nrm")
                    for sub in range(2):
                        sl = slice(sub * 512, (sub + 1) * 512)
                        cc = slice(h * 1024 + sub * 512, h * 1024 + (sub + 1) * 512)
                        for s in range(8):
                            nc.tensor.matmul(
                                nrm[:, sl], lhsT=ones_s[s], rhs=ads[s][:, cc],
                                start=(s == 0), stop=(s == 7),
                            )
                    nc.scalar.activation(
                        out=E[:, h * 1024:(h + 1) * 1024], in_=nrm[:],
                        func=mybir.ActivationFunctionType.Exp, scale=-1.0,
                    )
                Ev = E.rearrange("p (j hw) -> p hw j", j=N)
                nc.vector.tensor_reduce(
                    out=o_sb[:, hsl], in_=Ev,
                    axis=mybir.AxisListType.X, op=mybir.AluOpType.add,
                )
            nc.sync.dma_start(out=o_d[t], in_=o_sb)


# --------------------------------------------------------------------------
# host side
# --------------------------------------------------------------------------

def prep_inputs(x, T):
    """Shared (core-independent) device inputs."""
    xf = np.ascontiguousarray(x.reshape(N, HW, F))
    xT_np = np.ascontiguousarray(xf.transpose(2, 0, 1).reshape(F, N * HW))
    xT_in = xT_np.reshape(FH, 128, N * HW).astype(np.float16)
    tw = T.reshape(FH, 128, Q, 16, C).transpose(0, 2, 1, 3, 4)
    tw_in = np.ascontiguousarray(tw.reshape(FH, Q, 128, 128)).astype(np.float16)
    ones_in = np.zeros((8, 128, 128), np.float16)
    for s in range(8):
        for b in range(16):
            ones_in[s, b * 8:(b + 1) * 8, 16 * s + b] = 1.0
    return xT_np, xT_in, tw_in, ones_in


def core_in_map(xT_np, xT_in, tw_in, ones_in, k):
    xiT = np.ascontiguousarray(
        xT_np[:, k * NL * HW:(k + 1) * NL * HW]
    ).reshape(FH, 128, NL * HW).astype(np.float16)
    return {"xT": xT_in, "xiT": xiT, "tw": tw_in, "ones": ones_in}


def gather_ob(core_outs):
    """core_outs: list of 8 arrays [2,128,256] f32 -> o_b [N,16,16,B]."""
    obs = []
    for res in core_outs:
        v = res.reshape(2, 2, NL, 16, HW)          # t, q', i, b, hw
        obs.append(v.transpose(2, 4, 0, 1, 3).reshape(NL, HW, B))
    return np.concatenate(obs, axis=0).reshape(N, 16, 16, B)


_CACHED = {}


def _get_program(reps=1):
    key = ("nc", reps)
    if key in _CACHED:
        return _CACHED[key]
    import concourse.bacc as bacc
    import concourse.mybir as mybir
    import concourse.tile as tile

    nc = bacc.Bacc("TRN2", target_bir_lowering=False, debug=False,
                   num_devices=CORES)
    f16, f32 = mybir.dt.float16, mybir.dt.float32
    ins = {
        "xT": nc.dram_tensor("xT", [FH, 128, N * HW], f16, kind="ExternalInput").ap(),
        "xiT": nc.dram_tensor("xiT", [FH, 128, NL * HW], f16, kind="ExternalInput").ap(),
        "tw": nc.dram_tensor("tw", [FH, Q, 128, 128], f16, kind="ExternalInput").ap(),
        "ones": nc.dram_tensor("ones", [8, 128, 128], f16, kind="ExternalInput").ap(),
    }
    outs = {
        "o": nc.dram_tensor("o", [2, 128, HW], f32, kind="ExternalOutput").ap(),
    }
    with tile.TileContext(nc) as tc:
        for r in range(reps):
            build_body(tc, outs, ins, rep=r)
    nc.compile()
    _CACHED[key] = nc
    return nc


def kernel(x, T):
    x = np.asarray(x, dtype=np.float32)
    T = np.asarray(T, dtype=np.float32)
    from concourse.bass_utils import run_bass_kernel_spmd

    nc = _get_program()
    xT_np, xT_in, tw_in, ones_in = prep_inputs(x, T)
    in_maps = [core_in_map(xT_np, xT_in, tw_in, ones_in, k) for k in range(CORES)]
    res = run_bass_kernel_spmd(nc, in_maps, core_ids=list(range(CORES)))
    ob = gather_ob([r["o"] for r in res.results])
    return np.concatenate([x, ob], axis=3)
